# revision 22
# baseline (speedup 1.0000x reference)
"""Trainium2 Bass kernel for nn_CrossAttention_DenseAVInteractions.

Math: the reference attends over concat-pair tokens kv[(i,j)] = kv_v[i] + kv_a[j],
so scores additively factorize, the joint softmax over (i,j) is exactly the outer
product of two small softmaxes, and the context reduces to two tiny matmuls:

    S[q,(i,j)]   = scale*(q . Kv[i]) + scale*(q . Ka[j])
    attn[q,i,j]  = pv[q,i] * pa[q,j]        pv = softmax_i(scale q Kv^T),
                                            pa = softmax_j(scale q Ka^T)
    ctx[q]       = pv @ Vv + pa @ Va        (since rows of pv/pa sum to 1)
    out          = ctx @ Wproj + bproj

Per-core work (8 cores = 4 batches x 2 head-groups of 6 heads): project q/k/v for
its heads, softmax, materialize attn[6,196,12544] via a broadcast tensor_tensor
outer product on the vector engine, and compute the (partial) projected output.
Host sums the two head-group partials per batch and adds the bias.
"""

import numpy as np

import concourse.bacc as bacc
import concourse.mybir as mybir
from concourse import tile
from concourse import bass_utils
from concourse.masks import make_identity

F32 = mybir.dt.float32
AF = mybir.ActivationFunctionType
ALU = mybir.AluOpType
AXIS = mybir.AxisListType

B, NQ, NV, NA, D = 4, 196, 196, 64, 768
H, HD = 12, 64
HL = 6                    # heads per core
DH = HL * HD              # 384 head-group width
SCALE = (D // H) ** -0.5  # 0.125
NCORES = 8
KC = D // 128             # 6 contraction chunks of 128
W = NV * NA               # 12544 attn row width
IC = NV // 2              # 98 i's per attn column half
QCH = [(0, 128), (128, 68)]   # q-token chunks (start, len)

TRACE = False             # set True (e.g. from test.py) to neuron-profile
TRACE_KW = {}
LAST_RESULT = None

_cache = {}


def _softmax(nc, sm, s_ps, out_sb, rows, tag):
    """Row softmax: out_sb[:rows] = softmax(s_ps[:rows]) along the free dim."""
    mx = sm.tile([128, 1], F32, tag=f"mx{tag}")
    nc.vector.tensor_reduce(mx[:rows], s_ps[:rows], axis=AXIS.X, op=ALU.max,
                            negate=True)
    z = sm.tile([128, 1], F32, tag=f"z{tag}")
    nc.scalar.activation(out_sb[:rows], s_ps[:rows], AF.Exp, bias=mx[:rows],
                         scale=1.0, accum_out=z[:rows])
    rz = sm.tile([128, 1], F32, tag=f"rz{tag}")
    nc.vector.reciprocal(rz[:rows], z[:rows])
    nc.vector.tensor_scalar_mul(out_sb[:rows], out_sb[:rows], rz[:rows])


def _emit(nc, tc, ins, attn_o, out_o):
    with (
        tc.tile_pool(name="consts", bufs=1) as consts,
        tc.tile_pool(name="io", bufs=1) as io,
        tc.tile_pool(name="sm", bufs=4) as sm,
        tc.tile_pool(name="tpsb", bufs=2) as tpsb,
        tc.tile_pool(name="big", bufs=3) as big,
        tc.tile_pool(name="ps", bufs=5, space="PSUM") as ps,
        tc.tile_pool(name="pst", bufs=2, space="PSUM") as pst,
    ):
        ident = consts.tile([128, 128], F32)
        make_identity(nc, ident)

        # ---- load inputs. The host pre-shuffles everything to partition-major
        # [128, c, n] layout, so each load is one flat contiguous-per-partition
        # DMA. Ring split: score-path loads ride the SP ring ahead of the big
        # attn stores; late weights (ctx/proj path) + the small pv/pa repacks
        # ride the ACT ring so they never queue behind multi-MB transfers.
        def load3(name, eng):
            t = io.tile(list(ins[name].shape), F32, tag=name)
            eng.dma_start(t, ins[name])
            return t

        xmmT = load3("xmmT", nc.sync)
        wq = load3("wq", nc.sync)
        xvT = load3("xvT", nc.sync)
        wkc = load3("wkc", nc.sync)
        xaT = load3("xaT", nc.sync)
        wka = load3("wka", nc.sync)
        wvc = load3("wvc", nc.sync)
        wva = load3("wva", nc.sync)
        wp = load3("wp", nc.sync)

        # ---- projections (emitted lazily: head 0's score path first, so the
        # vector engine and the store stream start as early as possible)
        # qT/kvT/kaT in [col, token] layout: rows = local head cols (h*64+d)
        qT = io.tile([128, 3, NQ], F32, tag="qT")
        kvT = io.tile([128, 3, NQ], F32, tag="kvT")
        kaT = io.tile([128, 3, NA], F32, tag="kaT")

        def proj_m(m):
            cols = slice(m * 128, (m + 1) * 128)
            p1 = ps.tile([128, NQ], F32, tag="mm", name=f"pq{m}")
            p2 = ps.tile([128, NQ], F32, tag="mm", name=f"pk{m}")
            p3 = ps.tile([128, NA], F32, tag="mm", name=f"pa{m}")
            for k in range(KC):
                st, sp = (k == 0), (k == KC - 1)
                nc.tensor.matmul(p1, wq[:, k, cols], xmmT[:, k, :], start=st, stop=sp)
                nc.tensor.matmul(p2, wkc[:, k, cols], xvT[:, k, :], start=st, stop=sp)
                nc.tensor.matmul(p3, wka[:, k, cols], xaT[:, k, :], start=st, stop=sp)
            # fold the attention scale into q
            nc.scalar.activation(qT[:, m, :], p1, AF.Copy, bias=0.0, scale=SCALE)
            nc.scalar.copy(kvT[:, m, :], p2)
            nc.scalar.copy(kaT[:, m, :], p3)

        # Vv [i, d] and Va [j, d] in natural token-major layout
        vv = io.tile([128, 2, DH], F32, tag="vv")
        va = io.tile([64, DH], F32, tag="va")

        def proj_v():
            for mi, (q0, qn) in enumerate(QCH):
                p = ps.tile([128, DH], F32, tag="mm", name=f"pv{mi}")
                for k in range(KC):
                    nc.tensor.matmul(p[:qn], xvT[:, k, q0:q0 + qn], wvc[:, k, :],
                                     start=(k == 0), stop=(k == KC - 1))
                nc.scalar.copy(vv[:qn, mi, :], p[:qn])
            p = ps.tile([128, DH], F32, tag="mm", name="pva")
            for k in range(KC):
                nc.tensor.matmul(p[:NA], xaT[:, k, :], wva[:, k, :],
                                 start=(k == 0), stop=(k == KC - 1))
            nc.scalar.copy(va, p[:NA])

        # ---- per head: scores -> softmax -> repack to 128-row packed tiles
        # Packed layout: global attn row r = h*196 + q, tiled as 9 x 128 + 24.
        # All big stores then move 128 partitions (full DMA port parallelism).
        NT = (HL * NQ + 127) // 128          # 10
        trows = [min(128, HL * NQ - t * 128) for t in range(NT)]

        def spans(h, q0, qn):
            """(t, dst_po, src_off, len) covering rows h*196+[q0, q0+qn)."""
            g0 = h * NQ + q0
            out = []
            g = g0
            while g < g0 + qn:
                t = g // 128
                ln = min(128 * (t + 1), g0 + qn) - g
                out.append((t, g % 128, g - g0, ln))
                g += ln
            return out

        # how many (h, mi) blocks feed each packed tile
        need = [0] * NT
        for h in range(HL):
            for mi, (q0, qn) in enumerate(QCH):
                for t, _, _, _ in spans(h, q0, qn):
                    need[t] += 1

        pvp = [io.tile([128, NQ], F32, tag=f"pvp{t}", name=f"pvp{t}")
               for t in range(NT)]
        pap = [io.tile([128, NA], F32, tag=f"pap{t}", name=f"pap{t}")
               for t in range(NT)]

        def emit_attn_tile(t):
            """Outer product + store for one packed row tile."""
            rows = trows[t]
            for ci in range(2):
                at = big.tile([128, IC * NA], F32, tag="attn")
                nc.vector.tensor_mul(
                    at[:rows].rearrange("p (i j) -> p i j", j=NA),
                    pvp[t][:rows, ci * IC:(ci + 1) * IC, None].broadcast_to(
                        (rows, IC, NA)),
                    pap[t][:rows, None, :].broadcast_to((rows, IC, NA)),
                )
                nc.sync.dma_start(
                    attn_o[t * 128:t * 128 + rows,
                           ci * IC * NA:(ci + 1) * IC * NA], at[:rows])

        pv = {}
        pa = {}
        done = [0] * NT
        ctxT = io.tile([128, 3, NQ], F32, tag="ctxT")

        def score_block(h, mi):
            hc, ho = (h * HD) // 128, (h * HD) % 128
            q0, qn = QCH[mi]
            qh = qT[ho:ho + HD, hc, q0:q0 + qn]
            sv = ps.tile([128, NQ], F32, tag="mm", name=f"sv{h}_{mi}")
            nc.tensor.matmul(sv[:qn], qh, kvT[ho:ho + HD, hc, :],
                             start=True, stop=True)
            sa = ps.tile([128, NA], F32, tag="mm", name=f"sa{h}_{mi}")
            nc.tensor.matmul(sa[:qn], qh, kaT[ho:ho + HD, hc, :],
                             start=True, stop=True)
            pvt = io.tile([128, NQ], F32, tag=f"pv{h}_{mi}", name=f"pvt{h}_{mi}")
            pat = io.tile([128, NA], F32, tag=f"pa{h}_{mi}", name=f"pat{h}_{mi}")
            _softmax(nc, sm, sv, pvt, qn, "v")
            _softmax(nc, sm, sa, pat, qn, "a")
            pv[h, mi] = pvt
            pa[h, mi] = pat

            for t, po, so, ln in spans(h, q0, qn):
                nc.scalar.dma_start(pvp[t][po:po + ln, :], pvt[so:so + ln, :])
                nc.scalar.dma_start(pap[t][po:po + ln, :], pat[so:so + ln, :])
                done[t] += 1
                if done[t] == need[t]:
                    emit_attn_tile(t)

        def ctx_block(h):
            # transposes (PE) + context for this head
            pvT = tpsb.tile([128, 2, NQ], F32, tag="pvT", name=f"pvT{h}")
            paT = tpsb.tile([64, NQ], F32, tag="paT", name=f"paT{h}")
            for mi, (q0, qn) in enumerate(QCH):
                for ii, (i0, iN) in enumerate(QCH):
                    tp = pst.tile([128, 128], F32, tag="tp", name=f"tp{h}{mi}{ii}")
                    nc.tensor.transpose(tp[:iN, :qn],
                                        pv[h, mi][:qn, i0:i0 + iN],
                                        ident[:qn, :qn])
                    nc.scalar.copy(pvT[:iN, ii, q0:q0 + qn], tp[:iN, :qn])
                tp = pst.tile([128, 128], F32, tag="tp", name=f"tpa{h}{mi}")
                nc.tensor.transpose(tp[:NA, :qn], pa[h, mi][:qn, :],
                                    ident[:qn, :qn])
                nc.scalar.copy(paT[:, q0:q0 + qn], tp[:NA, :qn])

            hs = slice(h * HD, (h + 1) * HD)
            ct = ps.tile([64, NQ], F32, tag="mm", name=f"ct{h}")
            nc.tensor.matmul(ct, vv[:, 0, hs], pvT[:, 0, :], start=True, stop=False)
            nc.tensor.matmul(ct, vv[:68, 1, hs], pvT[:68, 1, :], start=False,
                             stop=False)
            nc.tensor.matmul(ct, va[:, hs], paT, start=False, stop=True)
            nc.scalar.copy(ctxT[(h % 2) * 64:(h % 2) * 64 + 64, h // 2, :], ct)

        # Emission order = rough schedule order: keep the score -> softmax ->
        # repack -> outer-product stream maximally ahead (it feeds the
        # bandwidth-bound attn stores); the ctx/proj path fills PE idle time
        # afterwards.
        proj_m(0)
        score_block(0, 0)
        score_block(0, 1)
        score_block(1, 0)
        score_block(1, 1)
        proj_m(1)
        score_block(2, 0)
        score_block(2, 1)
        score_block(3, 0)
        score_block(3, 1)
        proj_m(2)
        score_block(4, 0)
        score_block(4, 1)
        score_block(5, 0)
        score_block(5, 1)
        proj_v()
        for h in range(HL):
            ctx_block(h)

        # ---- output projection (partial over this head group)
        outp = io.tile([128, 2, D], F32, tag="outp")
        for mi, (q0, qn) in enumerate(QCH):
            for nI in range(2):
                op = ps.tile([128, DH], F32, tag="mm")
                for c in range(3):
                    nc.tensor.matmul(op[:qn], ctxT[:, c, q0:q0 + qn],
                                     wp[:, c, nI * DH:(nI + 1) * DH],
                                     start=(c == 0), stop=(c == 2))
                nc.scalar.copy(outp[:qn, mi, nI * DH:(nI + 1) * DH], op[:qn])
            nc.scalar.dma_start(out_o[q0:q0 + qn, :], outp[:qn, mi, :])


def _build():
    if "nc" in _cache:
        return _cache["nc"]
    nc = bacc.Bacc("TRN2", target_bir_lowering=False, debug=False,
                   enable_asserts=True, num_devices=NCORES)
    ins = {}
    for name, shape in [
        ("xmmT", (128, KC, NQ)), ("xvT", (128, KC, NV)), ("xaT", (128, KC, NA)),
        ("wq", (128, KC, DH)), ("wkc", (128, KC, DH)), ("wka", (128, KC, DH)),
        ("wvc", (128, KC, DH)), ("wva", (128, KC, DH)), ("wp", (128, 3, D)),
    ]:
        ins[name] = nc.dram_tensor(name, list(shape), F32,
                                   kind="ExternalInput").ap()
    attn_o = nc.dram_tensor("attn_o", [HL * NQ, W], F32,
                            kind="ExternalOutput").ap()
    out_o = nc.dram_tensor("out_o", [NQ, D], F32, kind="ExternalOutput").ap()
    with tile.TileContext(nc) as tc:
        _emit(nc, tc, ins, attn_o, out_o)
    nc.compile()
    _cache["nc"] = nc
    return nc


def _pshuf(a):
    """[(c*128), n] -> [128, c, n] partition-major, contiguous."""
    c = a.shape[0] // 128
    return np.ascontiguousarray(a.reshape(c, 128, -1).transpose(1, 0, 2))


def _shard(xmm, xa, xv, Wq, Wkv, Wproj):
    in_maps = []
    for core in range(NCORES):
        b, hg = core // 2, core % 2
        cs = slice(hg * DH, (hg + 1) * DH)
        vs = slice(D + hg * DH, D + (hg + 1) * DH)
        in_maps.append({
            "xmmT": _pshuf(xmm[b].T),
            "xvT": _pshuf(xv[b].T),
            "xaT": _pshuf(xa[b].T),
            "wq": _pshuf(Wq[:, cs]),
            "wkc": _pshuf(Wkv[:D, cs]),
            "wka": _pshuf(Wkv[D:, cs]),
            "wvc": _pshuf(Wkv[:D, vs]),
            "wva": _pshuf(Wkv[D:, vs]),
            "wp": _pshuf(Wproj[cs, :]),
        })
    return in_maps


def kernel(xmm, xa, xv, Wq, Wkv, Wproj, bproj):
    global LAST_RESULT
    xmm, xa, xv = (np.asarray(t, np.float32) for t in (xmm, xa, xv))
    Wq, Wkv, Wproj, bproj = (np.asarray(t, np.float32)
                             for t in (Wq, Wkv, Wproj, bproj))
    nc = _build()
    in_maps = _shard(xmm, xa, xv, Wq, Wkv, Wproj)
    res = bass_utils.run_bass_kernel_spmd(
        nc, in_maps, core_ids=list(range(NCORES)), trace=TRACE, **TRACE_KW)
    LAST_RESULT = res

    out = np.zeros((B, NQ, D), np.float32)
    attn = np.empty((B, H, NQ, W), np.float32)
    for core in range(NCORES):
        b, hg = core // 2, core % 2
        r = res.results[core]
        attn[b, hg * HL:(hg + 1) * HL] = r["attn_o"].reshape(HL, NQ, W)
        out[b] += r["out_o"]
    out += bproj.astype(np.float32)
    return out, attn


# revision 26
# speedup vs baseline: 1.1280x; 1.1280x over previous
"""Trainium2 Bass kernel for nn_CrossAttention_DenseAVInteractions.

Math: the reference attends over concat-pair tokens kv[(i,j)] = kv_v[i] + kv_a[j],
so scores additively factorize, the joint softmax over (i,j) is exactly the outer
product of two small softmaxes, and the context reduces to two tiny matmuls:

    S[q,(i,j)]   = scale*(q . Kv[i]) + scale*(q . Ka[j])
    attn[q,i,j]  = pv[q,i] * pa[q,j]        pv = softmax_i(scale q Kv^T),
                                            pa = softmax_j(scale q Ka^T)
    ctx[q]       = pv @ Vv + pa @ Va        (since rows of pv/pa sum to 1)
    out          = ctx @ Wproj + bproj

Per-core work (8 cores = 4 batches x 2 head-groups of 6 heads): project q/k/v for
its heads, softmax, materialize attn[6,196,12544] via a broadcast tensor_tensor
outer product on the vector engine, and compute the (partial) projected output.
Host sums the two head-group partials per batch and adds the bias.
"""

import numpy as np

import concourse.bacc as bacc
import concourse.mybir as mybir
from concourse import tile
from concourse import bass_utils
from concourse.masks import make_identity

F32 = mybir.dt.float32
AF = mybir.ActivationFunctionType
ALU = mybir.AluOpType
AXIS = mybir.AxisListType

B, NQ, NV, NA, D = 4, 196, 196, 64, 768
H, HD = 12, 64
HL = 6                    # heads per core
DH = HL * HD              # 384 head-group width
SCALE = (D // H) ** -0.5  # 0.125
NCORES = 8
KC = D // 128             # 6 contraction chunks of 128
W = NV * NA               # 12544 attn row width
IC = NV // 2              # 98 i's per attn column half
QCH = [(0, 128), (128, 68)]   # q-token chunks (start, len)

TRACE = False             # set True (e.g. from test.py) to neuron-profile
TRACE_KW = {}
LAST_RESULT = None

_cache = {}


def _softmax(nc, sm, s_ps, out_sb, rows, tag):
    """Row softmax: out_sb[:rows] = softmax(s_ps[:rows]) along the free dim."""
    mx = sm.tile([128, 1], F32, tag=f"mx{tag}")
    nc.vector.tensor_reduce(mx[:rows], s_ps[:rows], axis=AXIS.X, op=ALU.max,
                            negate=True)
    z = sm.tile([128, 1], F32, tag=f"z{tag}")
    nc.scalar.activation(out_sb[:rows], s_ps[:rows], AF.Exp, bias=mx[:rows],
                         scale=1.0, accum_out=z[:rows])
    rz = sm.tile([128, 1], F32, tag=f"rz{tag}")
    nc.vector.reciprocal(rz[:rows], z[:rows])
    nc.vector.tensor_scalar_mul(out_sb[:rows], out_sb[:rows], rz[:rows])


def _emit(nc, tc, ins, attn_o, out_o):
    with (
        tc.tile_pool(name="consts", bufs=1) as consts,
        tc.tile_pool(name="io", bufs=1) as io,
        tc.tile_pool(name="sm", bufs=4) as sm,
        tc.tile_pool(name="tpsb", bufs=2) as tpsb,
        tc.tile_pool(name="big", bufs=3) as big,
        tc.tile_pool(name="ps", bufs=5, space="PSUM") as ps,
        tc.tile_pool(name="pst", bufs=2, space="PSUM") as pst,
    ):
        ident = consts.tile([128, 128], F32)
        make_identity(nc, ident)

        # ---- load inputs. The host pre-shuffles everything to partition-major
        # [128, c, n] layout, so each load is one flat contiguous-per-partition
        # DMA. Ring split: score-path loads ride the SP ring ahead of the big
        # attn stores; late weights (ctx/proj path) + the small pv/pa repacks
        # ride the ACT ring so they never queue behind multi-MB transfers.
        def load3(name, eng):
            t = io.tile(list(ins[name].shape), F32, tag=name)
            eng.dma_start(t, ins[name])
            return t

        xmmT = load3("xmmT", nc.sync)
        wq = load3("wq", nc.sync)
        xvT = load3("xvT", nc.sync)
        wkc = load3("wkc", nc.sync)
        xaT = load3("xaT", nc.sync)
        wka = load3("wka", nc.sync)
        wvc = load3("wvc", nc.sync)
        wva = load3("wva", nc.sync)
        wp = load3("wp", nc.sync)

        # ---- projections (emitted lazily: head 0's score path first, so the
        # vector engine and the store stream start as early as possible)
        # qT/kvT/kaT in [col, token] layout: rows = local head cols (h*64+d)
        qT = io.tile([128, 3, NQ], F32, tag="qT")
        kvT = io.tile([128, 3, NQ], F32, tag="kvT")
        kaT = io.tile([128, 3, NA], F32, tag="kaT")

        def proj_m(m):
            cols = slice(m * 128, (m + 1) * 128)
            p1 = ps.tile([128, NQ], F32, tag="mm", name=f"pq{m}")
            p2 = ps.tile([128, NQ], F32, tag="mm", name=f"pk{m}")
            p3 = ps.tile([128, NA], F32, tag="mm", name=f"pa{m}")
            for k in range(KC):
                st, sp = (k == 0), (k == KC - 1)
                nc.tensor.matmul(p1, wq[:, k, cols], xmmT[:, k, :], start=st, stop=sp)
                nc.tensor.matmul(p2, wkc[:, k, cols], xvT[:, k, :], start=st, stop=sp)
                nc.tensor.matmul(p3, wka[:, k, cols], xaT[:, k, :], start=st, stop=sp)
            # fold the attention scale into q
            nc.scalar.activation(qT[:, m, :], p1, AF.Copy, bias=0.0, scale=SCALE)
            nc.scalar.copy(kvT[:, m, :], p2)
            nc.scalar.copy(kaT[:, m, :], p3)

        # Vv [i, d] and Va [j, d] in natural token-major layout
        vv = io.tile([128, 2, DH], F32, tag="vv")
        va = io.tile([64, DH], F32, tag="va")

        def proj_v():
            for mi, (q0, qn) in enumerate(QCH):
                p = ps.tile([128, DH], F32, tag="mm", name=f"pv{mi}")
                for k in range(KC):
                    nc.tensor.matmul(p[:qn], xvT[:, k, q0:q0 + qn], wvc[:, k, :],
                                     start=(k == 0), stop=(k == KC - 1))
                nc.scalar.copy(vv[:qn, mi, :], p[:qn])
            p = ps.tile([128, DH], F32, tag="mm", name="pva")
            for k in range(KC):
                nc.tensor.matmul(p[:NA], xaT[:, k, :], wva[:, k, :],
                                 start=(k == 0), stop=(k == KC - 1))
            nc.scalar.copy(va, p[:NA])

        # ---- per head: scores -> softmax -> repack to 128-row packed tiles
        # Packed layout: global attn row r = h*196 + q, tiled as 9 x 128 + 24.
        # All big stores then move 128 partitions (full DMA port parallelism).
        NT = (HL * NQ + 127) // 128          # 10
        trows = [min(128, HL * NQ - t * 128) for t in range(NT)]

        def spans(h, q0, qn):
            """(t, dst_po, src_off, len) covering rows h*196+[q0, q0+qn)."""
            g0 = h * NQ + q0
            out = []
            g = g0
            while g < g0 + qn:
                t = g // 128
                ln = min(128 * (t + 1), g0 + qn) - g
                out.append((t, g % 128, g - g0, ln))
                g += ln
            return out

        # how many (h, mi) blocks feed each packed tile
        need = [0] * NT
        for h in range(HL):
            for mi, (q0, qn) in enumerate(QCH):
                for t, _, _, _ in spans(h, q0, qn):
                    need[t] += 1

        # pv and pa packed side by side in one tile -> one repack DMA per span
        pvpap = [io.tile([128, NQ + NA], F32, tag=f"pvpap{t}", name=f"pvpap{t}")
                 for t in range(NT)]

        def emit_attn_tile(t):
            """Outer product + store for one packed row tile."""
            rows = trows[t]
            for ci in range(2):
                at = big.tile([128, IC * NA], F32, tag="attn")
                nc.vector.tensor_mul(
                    at[:rows].rearrange("p (i j) -> p i j", j=NA),
                    pvpap[t][:rows, ci * IC:(ci + 1) * IC, None].broadcast_to(
                        (rows, IC, NA)),
                    pvpap[t][:rows, None, NQ:NQ + NA].broadcast_to(
                        (rows, IC, NA)),
                )
                nc.sync.dma_start(
                    attn_o[t * 128:t * 128 + rows,
                           ci * IC * NA:(ci + 1) * IC * NA], at[:rows])

        pv = {}
        pa = {}
        done = [0] * NT
        ctxT = io.tile([128, 3, NQ], F32, tag="ctxT")

        ready = []  # completed packed tiles whose TT emission is deferred

        def score_block(h, mi):
            hc, ho = (h * HD) // 128, (h * HD) % 128
            q0, qn = QCH[mi]
            qh = qT[ho:ho + HD, hc, q0:q0 + qn]
            sv = ps.tile([128, NQ], F32, tag="mm", name=f"sv{h}_{mi}")
            nc.tensor.matmul(sv[:qn], qh, kvT[ho:ho + HD, hc, :],
                             start=True, stop=True)
            sa = ps.tile([128, NA], F32, tag="mm", name=f"sa{h}_{mi}")
            nc.tensor.matmul(sa[:qn], qh, kaT[ho:ho + HD, hc, :],
                             start=True, stop=True)
            pq = io.tile([128, NQ + NA], F32, tag=f"pq{h}_{mi}",
                         name=f"pq{h}_{mi}")
            _softmax(nc, sm, sv, pq[:, 0:NQ], qn, "v")
            _softmax(nc, sm, sa, pq[:, NQ:NQ + NA], qn, "a")
            pv[h, mi] = pq[:, 0:NQ]
            pa[h, mi] = pq[:, NQ:NQ + NA]

            # repacks first (they must beat the big stores into the global DMA
            # queue), then flush TTs for tiles completed by PREVIOUS blocks
            for t, po, so, ln in spans(h, q0, qn):
                nc.scalar.dma_start(pvpap[t][po:po + ln, :], pq[so:so + ln, :])
                done[t] += 1
                if done[t] == need[t]:
                    ready.append(t)
            while len(ready) > 1:
                emit_attn_tile(ready.pop(0))

        def ctx_block(h):
            # transposes (PE) + context for this head
            pvT = tpsb.tile([128, 2, NQ], F32, tag="pvT", name=f"pvT{h}")
            paT = tpsb.tile([64, NQ], F32, tag="paT", name=f"paT{h}")
            for mi, (q0, qn) in enumerate(QCH):
                for ii, (i0, iN) in enumerate(QCH):
                    tp = pst.tile([128, 128], F32, tag="tp", name=f"tp{h}{mi}{ii}")
                    nc.tensor.transpose(tp[:iN, :qn],
                                        pv[h, mi][:qn, i0:i0 + iN],
                                        ident[:qn, :qn])
                    nc.scalar.copy(pvT[:iN, ii, q0:q0 + qn], tp[:iN, :qn])
                tp = pst.tile([128, 128], F32, tag="tp", name=f"tpa{h}{mi}")
                nc.tensor.transpose(tp[:NA, :qn], pa[h, mi][:qn, :],
                                    ident[:qn, :qn])
                nc.scalar.copy(paT[:, q0:q0 + qn], tp[:NA, :qn])

            hs = slice(h * HD, (h + 1) * HD)
            ct = ps.tile([64, NQ], F32, tag="mm", name=f"ct{h}")
            nc.tensor.matmul(ct, vv[:, 0, hs], pvT[:, 0, :], start=True, stop=False)
            nc.tensor.matmul(ct, vv[:68, 1, hs], pvT[:68, 1, :], start=False,
                             stop=False)
            nc.tensor.matmul(ct, va[:, hs], paT, start=False, stop=True)
            nc.scalar.copy(ctxT[(h % 2) * 64:(h % 2) * 64 + 64, h // 2, :], ct)

        # Emission order = rough schedule order: keep the score -> softmax ->
        # repack -> outer-product stream maximally ahead (it feeds the
        # bandwidth-bound attn stores); the ctx/proj path fills PE idle time
        # afterwards.
        proj_m(0)
        score_block(0, 0)
        score_block(0, 1)
        score_block(1, 0)
        score_block(1, 1)
        proj_m(1)
        score_block(2, 0)
        score_block(2, 1)
        score_block(3, 0)
        score_block(3, 1)
        proj_m(2)
        score_block(4, 0)
        score_block(4, 1)
        score_block(5, 0)
        score_block(5, 1)
        while ready:
            emit_attn_tile(ready.pop(0))
        proj_v()
        for h in range(HL):
            ctx_block(h)

        # ---- output projection (partial over this head group)
        outp = io.tile([128, 2, D], F32, tag="outp")
        for mi, (q0, qn) in enumerate(QCH):
            for nI in range(2):
                op = ps.tile([128, DH], F32, tag="mm")
                for c in range(3):
                    nc.tensor.matmul(op[:qn], ctxT[:, c, q0:q0 + qn],
                                     wp[:, c, nI * DH:(nI + 1) * DH],
                                     start=(c == 0), stop=(c == 2))
                nc.scalar.copy(outp[:qn, mi, nI * DH:(nI + 1) * DH], op[:qn])
            nc.scalar.dma_start(out_o[q0:q0 + qn, :], outp[:qn, mi, :])


def _build():
    if "nc" in _cache:
        return _cache["nc"]
    nc = bacc.Bacc("TRN2", target_bir_lowering=False, debug=False,
                   enable_asserts=True, num_devices=NCORES)
    ins = {}
    for name, shape in [
        ("xmmT", (128, KC, NQ)), ("xvT", (128, KC, NV)), ("xaT", (128, KC, NA)),
        ("wq", (128, KC, DH)), ("wkc", (128, KC, DH)), ("wka", (128, KC, DH)),
        ("wvc", (128, KC, DH)), ("wva", (128, KC, DH)), ("wp", (128, 3, D)),
    ]:
        ins[name] = nc.dram_tensor(name, list(shape), F32,
                                   kind="ExternalInput").ap()
    attn_o = nc.dram_tensor("attn_o", [HL * NQ, W], F32,
                            kind="ExternalOutput").ap()
    out_o = nc.dram_tensor("out_o", [NQ, D], F32, kind="ExternalOutput").ap()
    with tile.TileContext(nc) as tc:
        _emit(nc, tc, ins, attn_o, out_o)
    nc.compile()
    _cache["nc"] = nc
    return nc


def _pshuf(a):
    """[(c*128), n] -> [128, c, n] partition-major, contiguous."""
    c = a.shape[0] // 128
    return np.ascontiguousarray(a.reshape(c, 128, -1).transpose(1, 0, 2))


def _shard(xmm, xa, xv, Wq, Wkv, Wproj):
    in_maps = []
    for core in range(NCORES):
        b, hg = core // 2, core % 2
        cs = slice(hg * DH, (hg + 1) * DH)
        vs = slice(D + hg * DH, D + (hg + 1) * DH)
        in_maps.append({
            "xmmT": _pshuf(xmm[b].T),
            "xvT": _pshuf(xv[b].T),
            "xaT": _pshuf(xa[b].T),
            "wq": _pshuf(Wq[:, cs]),
            "wkc": _pshuf(Wkv[:D, cs]),
            "wka": _pshuf(Wkv[D:, cs]),
            "wvc": _pshuf(Wkv[:D, vs]),
            "wva": _pshuf(Wkv[D:, vs]),
            "wp": _pshuf(Wproj[cs, :]),
        })
    return in_maps


def kernel(xmm, xa, xv, Wq, Wkv, Wproj, bproj):
    global LAST_RESULT
    xmm, xa, xv = (np.asarray(t, np.float32) for t in (xmm, xa, xv))
    Wq, Wkv, Wproj, bproj = (np.asarray(t, np.float32)
                             for t in (Wq, Wkv, Wproj, bproj))
    nc = _build()
    in_maps = _shard(xmm, xa, xv, Wq, Wkv, Wproj)
    res = bass_utils.run_bass_kernel_spmd(
        nc, in_maps, core_ids=list(range(NCORES)), trace=TRACE, **TRACE_KW)
    LAST_RESULT = res

    out = np.zeros((B, NQ, D), np.float32)
    attn = np.empty((B, H, NQ, W), np.float32)
    for core in range(NCORES):
        b, hg = core // 2, core % 2
        r = res.results[core]
        attn[b, hg * HL:(hg + 1) * HL] = r["attn_o"].reshape(HL, NQ, W)
        out[b] += r["out_o"]
    out += bproj.astype(np.float32)
    return out, attn
